# revision 2
# baseline (speedup 1.0000x reference)
"""Trainium2 Bass kernel for the gammatone-cochlea + LIF-SNN model.

Pipeline per core (32 of the 256 batch rows, pure data parallel):
  1. Gammatone conv [32ch, 64 taps] via tap-split Hankel matmuls (fp32 PE):
     4 batch rows per 128-partition group, block-diagonal lhsT, two
     accumulating matmuls per 512-sample block (taps 0-31 / 32-63, the
     second reading the same Hankel tile at free offset +32).
  2. ReLU on ScalarE (PSUM -> SBUF copy).
  3. Inner-hair-cell framing: DVE strided block-sums (128-sample blocks),
     env[t] = (S[t] + S[t+1]) / 256.
  4. AuditoryNerve: fused tensor_scalar (mult by per-partition scale,
     is_gt threshold) on a 4x partition-replicated env -> 320 spike rows.
  5. Bushy/IC/AC: batched fp32 matmuls for currents, then per-step LIF
     recurrences (beta=0.95, thr=1, subtract reset) on VectorE only.
Outputs [10, 124*32] per core; host reassembles to [B, T, 10].
"""
import numpy as np
import concourse.bass as bass
import concourse.bacc as bacc
import concourse.mybir as mybir
import concourse.tile as tile
from concourse.bass_utils import run_bass_kernel_spmd

dt = mybir.dt
AF = mybir.ActivationFunctionType
OP = mybir.AluOpType

NCORES = 8
B, N, C, K = 256, 16000, 32, 64
BLOC = B // NCORES            # 32 batch rows per core
WINDOW, STRIDE, T = 256, 128, 124
ANS, HID, OUT = 10, 50, 10
BETA, THR, AN_THR = 0.95, 1.0, 0.5
PAD_L, PAD_R = 31, 33         # SAME padding for K=64: 31 left, 32 right (+1 slack)
NPAD = PAD_L + N + PAD_R      # 16064
NBLK = 32                     # 31 x 512 + 1 x 128 = 16000 output samples
NSUM = 125                    # 128-sample block sums
FREE = T * BLOC               # 3968 (t-major, b-minor)
NGRP = BLOC // 4              # 8 groups of 4 rows

# jnp.linspace(0.5, 1.5, 10, dtype=f32), bitexact
_SCALES = np.array([0x3F000000, 0x3F1C71C7, 0x3F38E38E, 0x3F555555, 0x3F71C71D,
                    0x3F871C72, 0x3F955556, 0x3FA38E39, 0x3FB1C71D, 0x3FC00000],
                   dtype=np.uint32).view(np.float32)

_NC_CACHE = None


def _build_nc():
    nc = bacc.Bacc("TRN2", target_bir_lowering=False, debug=False,
                   num_devices=NCORES)

    apad = nc.dram_tensor("apad", [BLOC, NPAD], dt.float32, kind="ExternalInput")
    l1 = nc.dram_tensor("l1", [128, 128], dt.float32, kind="ExternalInput")
    l2 = nc.dram_tensor("l2", [128, 128], dt.float32, kind="ExternalInput")
    wb = nc.dram_tensor("wb", [3, 128, HID], dt.float32, kind="ExternalInput")
    wic = nc.dram_tensor("wic", [HID, HID], dt.float32, kind="ExternalInput")
    wac = nc.dram_tensor("wac", [HID, OUT], dt.float32, kind="ExternalInput")
    sv = nc.dram_tensor("sv", [128, 3], dt.float32, kind="ExternalInput")
    ospk = nc.dram_tensor("ospk", [OUT, FREE], dt.float32, kind="ExternalOutput")
    omem = nc.dram_tensor("omem", [OUT, FREE], dt.float32, kind="ExternalOutput")

    with tile.TileContext(nc) as tc:
        with tc.tile_pool(name="cpool", bufs=1) as cp:
            l1t = cp.tile([128, 128], dt.float32)
            nc.sync.dma_start(out=l1t[:, :], in_=l1[:, :])
            l2t = cp.tile([128, 128], dt.float32)
            nc.sync.dma_start(out=l2t[:, :], in_=l2[:, :])
            svt = cp.tile([128, 3], dt.float32)
            nc.sync.dma_start(out=svt[:, :], in_=sv[:, :])
            wbt = [cp.tile([128, HID], dt.float32, name=f"wbt{i}") for i in range(3)]
            for i in range(3):
                nc.sync.dma_start(out=wbt[i][:, :], in_=wb[i, :, :])
            wict = cp.tile([HID, HID], dt.float32)
            nc.sync.dma_start(out=wict[:, :], in_=wic[:, :])
            wact = cp.tile([HID, OUT], dt.float32)
            nc.sync.dma_start(out=wact[:, :], in_=wac[:, :])

            E4 = cp.tile([128, FREE], dt.float32)     # env, 4x partition-replicated
            S_all = cp.tile([128, NGRP * 126], dt.float32)
            env_all = cp.tile([128, NGRP * T], dt.float32)

            # ---------------- conv + framing ----------------
            with tc.tile_pool(name="hkp", bufs=4) as hkp, \
                 tc.tile_pool(name="ybp", bufs=4) as ybp, \
                 tc.tile_pool(name="psp", bufs=4, space="PSUM") as psp:
                for g in range(NGRP):
                    for i in range(NBLK):
                        w = 512 if i < 31 else 128
                        hk = hkp.tile([128, 544], dt.float32, tag="hk")
                        # Hankel: hk[r*32+k, j] = apad[4g+r, 512*i + j + k]
                        for r in range(4):
                            src = bass.AP(apad, (4 * g + r) * NPAD + 512 * i,
                                          [[1, 32], [1, w + 32]])
                            nc.sync.dma_start(out=hk[32 * r:32 * r + 32, 0:w + 32],
                                              in_=src)
                        acc = psp.tile([128, 512], dt.float32, tag="acc")
                        nc.tensor.matmul(acc[:, 0:w], l1t[:, :], hk[:, 0:w],
                                         start=True, stop=False)
                        nc.tensor.matmul(acc[:, 0:w], l2t[:, :], hk[:, 32:32 + w],
                                         start=False, stop=True)
                        yb = ybp.tile([128, 512], dt.float32, tag="yb")
                        nc.scalar.activation(yb[:, 0:w], acc[:, 0:w], AF.Relu)
                        # 128-sample block sums -> S_all[:, g*126 + 4i ...]
                        nblk = w // 128
                        view = bass.AP(yb.tensor, yb.offset,
                                       [list(yb.ap[0]), [128, nblk], [1, 128]])
                        nc.vector.tensor_reduce(
                            S_all[:, g * 126 + 4 * i: g * 126 + 4 * i + nblk],
                            view, axis=mybir.AxisListType.X, op=OP.add)
                    # env[t] = (S[t] + S[t+1]) * (1/256)
                    sg = g * 126
                    eg = g * T
                    nc.vector.tensor_tensor(env_all[:, eg:eg + T],
                                            S_all[:, sg:sg + T],
                                            S_all[:, sg + 1:sg + T + 1], OP.add)
                    nc.vector.tensor_scalar(env_all[:, eg:eg + T],
                                            env_all[:, eg:eg + T],
                                            1.0 / 256.0, None, OP.mult)

            # ---------------- shuffle env -> E4[c, t*32 + b] ----------------
            for g in range(NGRP):
                for r in range(4):
                    dst = bass.AP(E4.tensor, E4.offset + 4 * g + r,
                                  [list(E4.ap[0][:]), [BLOC, T]])
                    # only partitions 0:32 of E4
                    dst = bass.AP(E4.tensor, E4.offset + 4 * g + r,
                                  [[E4.ap[0][0], 32], [BLOC, T]])
                    src = env_all[32 * r:32 * r + 32, g * T:(g + 1) * T]
                    nc.sync.dma_start(out=dst, in_=src)
            for u in range(1, 4):
                nc.sync.dma_start(out=E4[32 * u:32 * u + 32, :], in_=E4[0:32, :])

            # ---------------- SNN ----------------
            NJ = 8
            FJ = FREE // NJ  # 496
            with tc.tile_pool(name="anp", bufs=2) as anp, \
                 tc.tile_pool(name="snn", bufs=1) as sp, \
                 tc.tile_pool(name="pss", bufs=1, space="PSUM") as pss:
                cur_b = sp.tile([HID, FREE], dt.float32)
                spk_b = sp.tile([HID, FREE], dt.float32)
                cur_ic = sp.tile([HID, FREE], dt.float32)
                spk_ic = sp.tile([HID, FREE], dt.float32)
                cur_ac = sp.tile([OUT, FREE], dt.float32)
                ospk_t = sp.tile([OUT, FREE], dt.float32)
                omem_t = sp.tile([OUT, FREE], dt.float32)
                memb = sp.tile([HID, BLOC], dt.float32)
                memic = sp.tile([HID, BLOC], dt.float32)
                z10 = sp.tile([OUT, BLOC], dt.float32)
                nc.vector.memset(memb[:, :], 0.0)
                nc.vector.memset(memic[:, :], 0.0)
                nc.vector.memset(z10[:, :], 0.0)

                # AuditoryNerve + bushy currents
                ps_b = [pss.tile([HID, FJ], dt.float32, name=f"psb{j}", tag=f"psb{j}")
                        for j in range(NJ)]
                for ch in range(3):
                    an = anp.tile([128, FREE], dt.float32, tag="an")
                    nc.vector.tensor_scalar(an[:, :], E4[:, :],
                                            svt[:, ch:ch + 1], AN_THR,
                                            OP.mult, OP.is_gt)
                    for j in range(NJ):
                        nc.tensor.matmul(ps_b[j][:, :], wbt[ch][:, :],
                                         an[:, j * FJ:(j + 1) * FJ],
                                         start=(ch == 0), stop=(ch == 2))
                for j in range(NJ):
                    nc.scalar.activation(cur_b[:, j * FJ:(j + 1) * FJ],
                                         ps_b[j][:, :], AF.Copy)

                def lif(mem, cur, spk_out, mem_out=None, zprev=None):
                    # 124 steps; writes spikes (and post-reset mem) slices
                    for t in range(T):
                        s = slice(t * BLOC, (t + 1) * BLOC)
                        if mem_out is None:
                            nc.vector.tensor_scalar(mem[:, :], mem[:, :], BETA,
                                                    None, OP.mult)
                            nc.vector.tensor_tensor(mem[:, :], mem[:, :],
                                                    cur[:, s], OP.add)
                            nc.vector.tensor_scalar(spk_out[:, s], mem[:, :], THR,
                                                    None, OP.is_gt)
                            nc.vector.tensor_tensor(mem[:, :], mem[:, :],
                                                    spk_out[:, s], OP.subtract)
                        else:
                            prev = zprev if t == 0 else mem_out[:, (t - 1) * BLOC:t * BLOC]
                            nc.vector.tensor_scalar(mem_out[:, s], prev, BETA,
                                                    None, OP.mult)
                            nc.vector.tensor_tensor(mem_out[:, s], mem_out[:, s],
                                                    cur[:, s], OP.add)
                            nc.vector.tensor_scalar(spk_out[:, s], mem_out[:, s],
                                                    THR, None, OP.is_gt)
                            nc.vector.tensor_tensor(mem_out[:, s], mem_out[:, s],
                                                    spk_out[:, s], OP.subtract)

                lif(memb, cur_b, spk_b)

                ps_i = [pss.tile([HID, FJ], dt.float32, name=f"psi{j}", tag=f"psb{j}")
                        for j in range(NJ)]
                for j in range(NJ):
                    nc.tensor.matmul(ps_i[j][:, :], wict[:, :],
                                     spk_b[:, j * FJ:(j + 1) * FJ],
                                     start=True, stop=True)
                    nc.scalar.activation(cur_ic[:, j * FJ:(j + 1) * FJ],
                                         ps_i[j][:, :], AF.Copy)

                lif(memic, cur_ic, spk_ic)

                ps_a = [pss.tile([OUT, FJ], dt.float32, name=f"psa{j}", tag=f"psb{j}")
                        for j in range(NJ)]
                for j in range(NJ):
                    nc.tensor.matmul(ps_a[j][:, :], wact[:, :],
                                     spk_ic[:, j * FJ:(j + 1) * FJ],
                                     start=True, stop=True)
                    nc.scalar.activation(cur_ac[:, j * FJ:(j + 1) * FJ],
                                         ps_a[j][:, :], AF.Copy)

                lif(None, cur_ac, ospk_t, mem_out=omem_t, zprev=z10[:, :])

                nc.sync.dma_start(out=ospk[:, :], in_=ospk_t[:, :])
                nc.sync.dma_start(out=omem[:, :], in_=omem_t[:, :])

    nc.finalize()
    return nc


def _prep_inputs(audio, gt_kernels, W_bushy, W_ic, W_ac):
    audio = np.ascontiguousarray(audio, dtype=np.float32)
    gt = np.ascontiguousarray(gt_kernels, dtype=np.float32)
    Wb = np.ascontiguousarray(W_bushy, dtype=np.float32)

    l1 = np.zeros((128, 128), np.float32)
    l2 = np.zeros((128, 128), np.float32)
    for r in range(4):
        # lhsT[r*32+k, r*32+c] = gt[c, k]
        l1[r * 32:r * 32 + 32, r * 32:r * 32 + 32] = gt[:, 0:32].T
        l2[r * 32:r * 32 + 32, r * 32:r * 32 + 32] = gt[:, 32:64].T

    wb = np.zeros((3, 128, HID), np.float32)
    sv = np.zeros((128, 3), np.float32)
    for ch in range(3):
        for u in range(4):
            a = ch * 4 + u
            if a >= ANS:
                continue
            # wb[ch, u*32+c, h] = W_bushy[h, c*10 + a]
            wb[ch, u * 32:u * 32 + 32, :] = Wb[:, a::ANS].T
            sv[u * 32:u * 32 + 32, ch] = _SCALES[a]
    wic = np.ascontiguousarray(W_ic.T, dtype=np.float32)
    wac = np.ascontiguousarray(W_ac.T, dtype=np.float32)

    in_maps = []
    for c in range(NCORES):
        rows = audio[c * BLOC:(c + 1) * BLOC]
        apad = np.zeros((BLOC, NPAD), np.float32)
        apad[:, PAD_L:PAD_L + N] = rows
        in_maps.append({"apad": apad, "l1": l1, "l2": l2, "wb": wb,
                        "wic": wic, "wac": wac, "sv": sv})
    return in_maps


def kernel(audio, gt_kernels, W_bushy, W_ic, W_ac, _trace=False):
    global _NC_CACHE
    if _NC_CACHE is None:
        _NC_CACHE = _build_nc()
    nc = _NC_CACHE
    in_maps = _prep_inputs(audio, gt_kernels, W_bushy, W_ic, W_ac)
    res = run_bass_kernel_spmd(nc, in_maps, core_ids=list(range(NCORES)),
                               trace=_trace)
    spk = np.empty((B, T, OUT), np.float32)
    mem = np.empty((B, T, OUT), np.float32)
    for c in range(NCORES):
        # [o, t*32+b] -> [b, t, o]
        spk[c * BLOC:(c + 1) * BLOC] = (
            res.results[c]["ospk"].reshape(OUT, T, BLOC).transpose(2, 1, 0))
        mem[c * BLOC:(c + 1) * BLOC] = (
            res.results[c]["omem"].reshape(OUT, T, BLOC).transpose(2, 1, 0))
    if _trace:
        kernel._last_results = res
    return spk, mem


# revision 22
# speedup vs baseline: 1.4349x; 1.4349x over previous
"""Trainium2 Bass kernel for the gammatone-cochlea + LIF-SNN model.

Pipeline per core (32 of the 256 batch rows, pure data parallel):
  1. Gammatone conv [32ch, 64 taps] via tap-split Hankel matmuls (fp32 PE):
     4 batch rows per 128-partition group, block-diagonal lhsT, two
     accumulating matmuls per 512-sample block (taps 0-31 / 32-63, the
     second reading the same Hankel tile at free offset +32).
  2. ReLU on ScalarE (PSUM -> SBUF copy).
  3. Inner-hair-cell framing: DVE strided block-sums (128-sample blocks),
     env[t] = (S[t] + S[t+1]) / 256.
  4. AuditoryNerve: fused tensor_scalar (mult by per-partition scale,
     is_gt threshold) on a 4x partition-replicated env -> 320 spike rows.
  5. Bushy/IC/AC: batched fp32 matmuls for currents, then per-step LIF
     recurrences (beta=0.95, thr=1, subtract reset) on VectorE only.
     The SNN runs as two batch halves; half A is interleaved under the
     conv of groups 4-7 to hide its serial LIF chain.
Outputs [10, 124*32] per core; host reassembles to [B, T, 10].
"""
import numpy as np
import concourse.bass as bass
import concourse.bacc as bacc
import concourse.mybir as mybir
import concourse.tile as tile
from concourse.bass_utils import run_bass_kernel_spmd

dt = mybir.dt
AF = mybir.ActivationFunctionType
OP = mybir.AluOpType

NCORES = 8
B, N, C, K = 256, 16000, 32, 64
BLOC = B // NCORES            # 32 batch rows per core
WINDOW, STRIDE, T = 256, 128, 124
ANS, HID, OUT = 10, 50, 10
BETA, THR, AN_THR = 0.95, 1.0, 0.5
PAD_L, PAD_R = 31, 33         # SAME padding for K=64: 31 left, 32 right (+1 slack)
NPAD = PAD_L + N + PAD_R      # 16064
FREE = T * BLOC               # 3968 (t-major, b-minor)
NGRP = BLOC // 4              # 8 groups of 4 rows
STRIPS = [2048] * 7 + [1664]  # 4-block strips per group

# jnp.linspace(0.5, 1.5, 10, dtype=f32), bitexact
_SCALES = np.array([0x3F000000, 0x3F1C71C7, 0x3F38E38E, 0x3F555555, 0x3F71C71D,
                    0x3F871C72, 0x3F955556, 0x3FA38E39, 0x3FB1C71D, 0x3FC00000],
                   dtype=np.uint32).view(np.float32)

_NC_CACHE = None


def _build_nc():
    nc = bacc.Bacc("TRN2", target_bir_lowering=False, debug=False,
                   num_devices=NCORES)

    apad = nc.dram_tensor("apad", [BLOC, NPAD], dt.float32, kind="ExternalInput")
    l1 = nc.dram_tensor("l1", [128, 128], dt.float32, kind="ExternalInput")
    l2 = nc.dram_tensor("l2", [128, 128], dt.float32, kind="ExternalInput")
    wb = nc.dram_tensor("wb", [3, 128, HID], dt.float32, kind="ExternalInput")
    wic = nc.dram_tensor("wic", [HID, HID], dt.float32, kind="ExternalInput")
    wac = nc.dram_tensor("wac", [HID, OUT], dt.float32, kind="ExternalInput")
    sv = nc.dram_tensor("sv", [128, 3], dt.float32, kind="ExternalInput")
    selr = nc.dram_tensor("selr", [4, 128, 128], dt.float32, kind="ExternalInput")
    ospk = nc.dram_tensor("ospk", [OUT, FREE], dt.float32, kind="ExternalOutput")
    omem = nc.dram_tensor("omem", [OUT, FREE], dt.float32, kind="ExternalOutput")

    with tile.TileContext(nc) as tc:
        with tc.tile_pool(name="cpool", bufs=1) as cp:
            l1t = cp.tile([128, 128], dt.float32)
            nc.sync.dma_start(out=l1t[:, :], in_=l1[:, :])
            l2t = cp.tile([128, 128], dt.float32)
            nc.sync.dma_start(out=l2t[:, :], in_=l2[:, :])
            svt = cp.tile([128, 3], dt.float32)
            nc.sync.dma_start(out=svt[:, :], in_=sv[:, :])
            wbt = [cp.tile([128, HID], dt.float32, name=f"wbt{i}") for i in range(3)]
            for i in range(3):
                nc.sync.dma_start(out=wbt[i][:, :], in_=wb[i, :, :])
            wict = cp.tile([HID, HID], dt.float32)
            nc.sync.dma_start(out=wict[:, :], in_=wic[:, :])
            wact = cp.tile([HID, OUT], dt.float32)
            nc.sync.dma_start(out=wact[:, :], in_=wac[:, :])
            selt = [cp.tile([128, 128], dt.float32, name=f"selt{r}")
                    for r in range(4)]
            for r in range(4):
                nc.sync.dma_start(out=selt[r][:, :], in_=selr[r, :, :])

            GF = 496             # per-group free = 4*124
            E4 = cp.tile([128, FREE], dt.float32)     # env, 4x partition-replicated
            S_all = cp.tile([128, NGRP * 126], dt.float32)
            env_all = cp.tile([128, NGRP * T], dt.float32)
            ospk_t = cp.tile([OUT, FREE], dt.float32,
                             padded_shape=[OUT, FREE + 32])
            omem_t = cp.tile([OUT, FREE], dt.float32,
                             padded_shape=[OUT, FREE + 32])
            z10 = cp.tile([OUT, 16], dt.float32)
            nc.vector.memset(z10[:, :], 0.0)

            hkp = tc.alloc_tile_pool(name="hkp", bufs=5)
            ybp = tc.alloc_tile_pool(name="ybp", bufs=12)
            anp = tc.alloc_tile_pool(name="anp", bufs=2)
            sp = tc.alloc_tile_pool(name="snn", bufs=1)
            pss = tc.alloc_tile_pool(name="pss", bufs=1, space="PSUM")
            psp = tc.alloc_tile_pool(name="psp", bufs=1, space="PSUM")

            def conv_group(g):
                """Generator: conv + framing for rows 4g..4g+4; yields per strip."""
                for si, sw in enumerate(STRIPS):
                    s0 = 2048 * si
                    hk = hkp.tile([128, 2112], dt.float32, tag="hk", name="hk")
                    # Hankel: hk[r*32+k, j] = apad[4g+r, s0 + j + k]
                    for r in range(4):
                        srcr = bass.AP(apad, (4 * g + r) * NPAD + s0,
                                       [[1, 32], [1, sw + 32]])
                        nc.sync.dma_start(out=hk[32 * r:32 * r + 32, 0:sw + 32],
                                          in_=srcr)
                    nb4 = (sw + 511) // 512
                    accs = []
                    for b4 in range(nb4):
                        w = min(512, sw - 512 * b4)
                        acc = psp.tile([128, 512], dt.float32, tag=f"acc{b4}",
                                       name="acc")
                        accs.append((acc, w))
                        nc.tensor.matmul(acc[:, 0:w], l1t[:, :],
                                         hk[:, 512 * b4:512 * b4 + w],
                                         start=True, stop=False)
                    for b4 in range(nb4):
                        acc, w = accs[b4]
                        nc.tensor.matmul(acc[:, 0:w], l2t[:, :],
                                         hk[:, 512 * b4 + 32:512 * b4 + 32 + w],
                                         start=False, stop=True)
                    for b4 in range(nb4):
                        acc, w = accs[b4]
                        yb = ybp.tile([128, 512], dt.float32, tag="yb", name="yb")
                        nc.scalar.activation(yb[:, 0:w], acc[:, 0:w], AF.Relu)
                        nblk = w // 128
                        i = 4 * si + b4
                        view = bass.AP(yb.tensor, yb.offset,
                                       [list(yb.ap[0]), [128, nblk], [1, 128]])
                        nc.vector.tensor_reduce(
                            S_all[:, g * 126 + 4 * i: g * 126 + 4 * i + nblk],
                            view, axis=mybir.AxisListType.X, op=OP.add)
                    yield

            def conv_epilogue(g):
                # env[t] = (S[t] + S[t+1]) * (1/256)
                sg = g * 126
                eg = g * T
                nc.vector.tensor_tensor(env_all[:, eg:eg + T],
                                        S_all[:, sg:sg + T],
                                        S_all[:, sg + 1:sg + T + 1], OP.add)
                nc.vector.tensor_scalar(env_all[:, eg:eg + T],
                                        env_all[:, eg:eg + T],
                                        1.0 / 256.0, None, OP.mult)
                # shuffle+replicate env -> E4[u*32+c, (4g+r)*124 + t]
                # via 0/1 selector matmuls (no DMA in the dependency chain)
                shf = pss.tile([128, GF], dt.float32, tag="misc", bufs=2, name="shf")
                for r in range(4):
                    nc.tensor.matmul(shf[:, r * T:(r + 1) * T], selt[r][:, :],
                                     env_all[:, eg:eg + T],
                                     start=True, stop=True)
                nc.scalar.activation(E4[:, g * GF:(g + 1) * GF], shf[:, :],
                                     AF.Copy)

            def an_group(g):
                """AN + bushy currents for group g (columns g*496..)."""
                sl = slice(g * GF, (g + 1) * GF)
                ps_cb = pss.tile([HID, GF], dt.float32, tag="misc", bufs=2, name="ps_cb")
                for ch in range(3):
                    an = anp.tile([128, GF], dt.float32, tag="an", name="an")
                    nc.vector.tensor_scalar(an[:, :], E4[:, sl],
                                            svt[:, ch:ch + 1], AN_THR,
                                            OP.mult, OP.is_gt)
                    nc.tensor.matmul(ps_cb[:, :], wbt[ch][:, :], an[:, :],
                                     start=(ch == 0), stop=(ch == 2))
                nc.scalar.activation(cur_b[:, sl], ps_cb[:, :], AF.Copy)

            cur_b = cp.tile([HID, FREE], dt.float32,
                            padded_shape=[HID, FREE + 32])

            # -------- conv + AN driver (AN one group late to hide deps) --------
            for g in range(NGRP):
                for _ in conv_group(g):
                    pass
                if g >= 1:
                    an_group(g - 1)
                conv_epilogue(g)
            an_group(NGRP - 1)
            psp.release()

            # -------- wavefront LIF: bushy(t), ic(t-1), ac(t-2) --------
            # free layout is b-major: column b*124 + t; a time-slice is
            # a stride-124 AP of 32 columns.
            def tsl(ap2d, t):
                return bass.AP(ap2d.tensor, ap2d.offset + t,
                               [list(ap2d.ap[0]), [T, BLOC]])

            memb = sp.tile([HID, BLOC], dt.float32)
            memic = sp.tile([HID, BLOC], dt.float32)
            nc.vector.memset(memb[:, :], 0.0)
            nc.vector.memset(memic[:, :], 0.0)
            z10 = sp.tile([OUT, BLOC], dt.float32)
            nc.vector.memset(z10[:, :], 0.0)

            spk_b_t = {}
            spk_ic_t = {}
            cur_ic_t = {}
            cur_ac_t = {}

            def ic_mm(t):
                sb = spk_b_t.pop(t)
                pi = pss.tile([HID, BLOC], dt.float32, tag="pp", bufs=2,
                              name="pic")
                nc.tensor.matmul(pi[:, :], wict[:, :], sb[:, :],
                                 start=True, stop=True)
                ci = sp.tile([HID, BLOC], dt.float32, tag="cit", bufs=4,
                             name="cit")
                cur_ic_t[t] = ci
                nc.scalar.activation(ci[:, :], pi[:, :], AF.Copy)

            def ac_mm(t):
                si = spk_ic_t.pop(t)
                pa = pss.tile([OUT, BLOC], dt.float32, tag="pp", bufs=2,
                              name="pac")
                nc.tensor.matmul(pa[:, :], wact[:, :], si[:, :],
                                 start=True, stop=True)
                ca = sp.tile([OUT, BLOC], dt.float32, tag="cat", bufs=4,
                             name="cat")
                cur_ac_t[t] = ca
                nc.scalar.activation(ca[:, :], pa[:, :], AF.Copy)

            # interleave the three chains op-by-op so adjacent DVE ops are
            # from different (independent) chains
            def chain_steps(fns):
                its = [iter(f) for f in fns]
                done = [False] * len(its)
                while not all(done):
                    for k, it in enumerate(its):
                        if not done[k]:
                            try:
                                next(it)
                            except StopIteration:
                                done[k] = True

            def bushy_chain():
                for t in range(T):
                    nc.vector.tensor_scalar(memb[:, :], memb[:, :], BETA,
                                            None, OP.mult)
                    yield
                    nc.vector.tensor_tensor(memb[:, :], memb[:, :],
                                            tsl(cur_b, t), OP.add)
                    yield
                    sb = sp.tile([HID, BLOC], dt.float32, tag="sbt", bufs=4,
                                 name="sbt")
                    spk_b_t[t] = sb
                    nc.vector.tensor_scalar(sb[:, :], memb[:, :], THR,
                                            None, OP.is_gt)
                    ic_mm(t)
                    yield
                    nc.vector.tensor_tensor(memb[:, :], memb[:, :], sb[:, :],
                                            OP.subtract)
                    yield

            def ic_chain():
                yield  # offset by one wavefront step
                for _ in range(4):
                    yield
                for t in range(T):
                    ci = cur_ic_t.pop(t)
                    nc.vector.tensor_scalar(memic[:, :], memic[:, :], BETA,
                                            None, OP.mult)
                    yield
                    nc.vector.tensor_tensor(memic[:, :], memic[:, :],
                                            ci[:, :], OP.add)
                    yield
                    si = sp.tile([HID, BLOC], dt.float32, tag="sit", bufs=4,
                                 name="sit")
                    spk_ic_t[t] = si
                    nc.vector.tensor_scalar(si[:, :], memic[:, :], THR,
                                            None, OP.is_gt)
                    ac_mm(t)
                    yield
                    nc.vector.tensor_tensor(memic[:, :], memic[:, :],
                                            si[:, :], OP.subtract)
                    yield

            def ac_chain():
                for _ in range(8):
                    yield
                for t in range(T):
                    ca = cur_ac_t.pop(t)
                    prev = z10[:, :] if t == 0 else tsl(omem_t, t - 1)
                    nc.vector.tensor_scalar(tsl(omem_t, t), prev, BETA,
                                            None, OP.mult)
                    yield
                    nc.vector.tensor_tensor(tsl(omem_t, t), tsl(omem_t, t),
                                            ca[:, :], OP.add)
                    yield
                    nc.vector.tensor_scalar(tsl(ospk_t, t), tsl(omem_t, t),
                                            THR, None, OP.is_gt)
                    yield
                    nc.vector.tensor_tensor(tsl(omem_t, t), tsl(omem_t, t),
                                            tsl(ospk_t, t), OP.subtract)
                    yield

            chain_steps([bushy_chain(), ic_chain(), ac_chain()])

            nc.sync.dma_start(out=ospk[:, :], in_=ospk_t[:, :])
            nc.sync.dma_start(out=omem[:, :], in_=omem_t[:, :])

            pss.release()
            sp.release()
            anp.release()
            ybp.release()
            hkp.release()

    nc.finalize()
    return nc


def _prep_inputs(audio, gt_kernels, W_bushy, W_ic, W_ac):
    audio = np.ascontiguousarray(audio, dtype=np.float32)
    gt = np.ascontiguousarray(gt_kernels, dtype=np.float32)
    Wb = np.ascontiguousarray(W_bushy, dtype=np.float32)

    l1 = np.zeros((128, 128), np.float32)
    l2 = np.zeros((128, 128), np.float32)
    for r in range(4):
        # lhsT[r*32+k, r*32+c] = gt[c, k]
        l1[r * 32:r * 32 + 32, r * 32:r * 32 + 32] = gt[:, 0:32].T
        l2[r * 32:r * 32 + 32, r * 32:r * 32 + 32] = gt[:, 32:64].T

    wb = np.zeros((3, 128, HID), np.float32)
    sv = np.zeros((128, 3), np.float32)
    for ch in range(3):
        for u in range(4):
            a = ch * 4 + u
            if a >= ANS:
                continue
            # wb[ch, u*32+c, h] = W_bushy[h, c*10 + a]
            wb[ch, u * 32:u * 32 + 32, :] = Wb[:, a::ANS].T
            sv[u * 32:u * 32 + 32, ch] = _SCALES[a]
    selr = np.zeros((4, 128, 128), np.float32)
    for r in range(4):
        for u in range(4):
            for c in range(32):
                selr[r, r * 32 + c, u * 32 + c] = 1.0
    wic = np.ascontiguousarray(W_ic.T, dtype=np.float32)
    wac = np.ascontiguousarray(W_ac.T, dtype=np.float32)

    in_maps = []
    for c in range(NCORES):
        rows = audio[c * BLOC:(c + 1) * BLOC]
        apad = np.zeros((BLOC, NPAD), np.float32)
        apad[:, PAD_L:PAD_L + N] = rows
        in_maps.append({"apad": apad, "l1": l1, "l2": l2, "wb": wb,
                        "wic": wic, "wac": wac, "sv": sv, "selr": selr})
    return in_maps


def kernel(audio, gt_kernels, W_bushy, W_ic, W_ac, _trace=False):
    global _NC_CACHE
    if _NC_CACHE is None:
        _NC_CACHE = _build_nc()
    nc = _NC_CACHE
    in_maps = _prep_inputs(audio, gt_kernels, W_bushy, W_ic, W_ac)
    res = run_bass_kernel_spmd(nc, in_maps, core_ids=list(range(NCORES)),
                               trace=_trace)
    spk = np.empty((B, T, OUT), np.float32)
    mem = np.empty((B, T, OUT), np.float32)
    for c in range(NCORES):
        # [o, b*124+t] -> [b, t, o]
        spk[c * BLOC:(c + 1) * BLOC] = (
            res.results[c]["ospk"].reshape(OUT, BLOC, T).transpose(1, 2, 0))
        mem[c * BLOC:(c + 1) * BLOC] = (
            res.results[c]["omem"].reshape(OUT, BLOC, T).transpose(1, 2, 0))
    kernel._last_results = res
    return spk, mem


# revision 23
# speedup vs baseline: 1.4598x; 1.0173x over previous
"""Trainium2 Bass kernel for the gammatone-cochlea + LIF-SNN model.

Pipeline per core (32 of the 256 batch rows, pure data parallel):
  1. Gammatone conv [32ch, 64 taps] via tap-split Hankel matmuls (fp32 PE):
     4 batch rows per 128-partition group, block-diagonal lhsT, two
     accumulating matmuls per 512-sample block (taps 0-31 / 32-63, the
     second reading the same Hankel tile at free offset +32).
  2. ReLU on ScalarE (PSUM -> SBUF copy).
  3. Inner-hair-cell framing: DVE strided block-sums (128-sample blocks),
     env[t] = (S[t] + S[t+1]) / 256.
  4. AuditoryNerve: fused tensor_scalar (mult by per-partition scale,
     is_gt threshold) on a 4x partition-replicated env -> 320 spike rows.
  5. Bushy/IC/AC: batched fp32 matmuls for currents, then per-step LIF
     recurrences (beta=0.95, thr=1, subtract reset) on VectorE only.
     The SNN runs as two batch halves; half A is interleaved under the
     conv of groups 4-7 to hide its serial LIF chain.
Outputs [10, 124*32] per core; host reassembles to [B, T, 10].
"""
import numpy as np
import concourse.bass as bass
import concourse.bacc as bacc
import concourse.mybir as mybir
import concourse.tile as tile
from concourse.bass_utils import run_bass_kernel_spmd

dt = mybir.dt
AF = mybir.ActivationFunctionType
OP = mybir.AluOpType

NCORES = 8
B, N, C, K = 256, 16000, 32, 64
BLOC = B // NCORES            # 32 batch rows per core
WINDOW, STRIDE, T = 256, 128, 124
ANS, HID, OUT = 10, 50, 10
BETA, THR, AN_THR = 0.95, 1.0, 0.5
PAD_L, PAD_R = 31, 33         # SAME padding for K=64: 31 left, 32 right (+1 slack)
NPAD = PAD_L + N + PAD_R      # 16064
FREE = T * BLOC               # 3968 (t-major, b-minor)
NGRP = BLOC // 4              # 8 groups of 4 rows
STRIPS = [2048] * 7 + [1664]  # 4-block strips per group

# jnp.linspace(0.5, 1.5, 10, dtype=f32), bitexact
_SCALES = np.array([0x3F000000, 0x3F1C71C7, 0x3F38E38E, 0x3F555555, 0x3F71C71D,
                    0x3F871C72, 0x3F955556, 0x3FA38E39, 0x3FB1C71D, 0x3FC00000],
                   dtype=np.uint32).view(np.float32)

_NC_CACHE = None


def _build_nc():
    nc = bacc.Bacc("TRN2", target_bir_lowering=False, debug=False,
                   num_devices=NCORES)

    apad = nc.dram_tensor("apad", [BLOC, NPAD], dt.float32, kind="ExternalInput")
    l1 = nc.dram_tensor("l1", [128, 128], dt.float32, kind="ExternalInput")
    l2 = nc.dram_tensor("l2", [128, 128], dt.float32, kind="ExternalInput")
    wb = nc.dram_tensor("wb", [3, 128, HID], dt.float32, kind="ExternalInput")
    wic = nc.dram_tensor("wic", [HID, HID], dt.float32, kind="ExternalInput")
    wac = nc.dram_tensor("wac", [HID, OUT], dt.float32, kind="ExternalInput")
    sv = nc.dram_tensor("sv", [128, 3], dt.float32, kind="ExternalInput")
    selr = nc.dram_tensor("selr", [4, 128, 128], dt.float32, kind="ExternalInput")
    ospk = nc.dram_tensor("ospk", [OUT, FREE], dt.float32, kind="ExternalOutput")
    omem = nc.dram_tensor("omem", [OUT, FREE], dt.float32, kind="ExternalOutput")

    with tile.TileContext(nc) as tc:
        with tc.tile_pool(name="cpool", bufs=1) as cp:
            l1t = cp.tile([128, 128], dt.float32)
            nc.sync.dma_start(out=l1t[:, :], in_=l1[:, :])
            l2t = cp.tile([128, 128], dt.float32)
            nc.sync.dma_start(out=l2t[:, :], in_=l2[:, :])
            svt = cp.tile([128, 3], dt.float32)
            nc.gpsimd.dma_start(out=svt[:, :], in_=sv[:, :])
            wbt = [cp.tile([128, HID], dt.float32, name=f"wbt{i}") for i in range(3)]
            for i in range(3):
                nc.gpsimd.dma_start(out=wbt[i][:, :], in_=wb[i, :, :])
            wict = cp.tile([HID, HID], dt.float32)
            nc.gpsimd.dma_start(out=wict[:, :], in_=wic[:, :])
            wact = cp.tile([HID, OUT], dt.float32)
            nc.gpsimd.dma_start(out=wact[:, :], in_=wac[:, :])
            selt = [cp.tile([128, 128], dt.float32, name=f"selt{r}")
                    for r in range(4)]
            for r in range(4):
                nc.scalar.dma_start(out=selt[r][:, :], in_=selr[r, :, :])

            GF = 496             # per-group free = 4*124
            E4 = cp.tile([128, FREE], dt.float32)     # env, 4x partition-replicated
            S_all = cp.tile([128, NGRP * 126], dt.float32)
            env_all = cp.tile([128, NGRP * T], dt.float32)
            ospk_t = cp.tile([OUT, FREE], dt.float32,
                             padded_shape=[OUT, FREE + 32])
            omem_t = cp.tile([OUT, FREE], dt.float32,
                             padded_shape=[OUT, FREE + 32])
            z10 = cp.tile([OUT, 16], dt.float32)
            nc.vector.memset(z10[:, :], 0.0)

            hkp = tc.alloc_tile_pool(name="hkp", bufs=5)
            ybp = tc.alloc_tile_pool(name="ybp", bufs=12)
            anp = tc.alloc_tile_pool(name="anp", bufs=2)
            sp = tc.alloc_tile_pool(name="snn", bufs=1)
            pss = tc.alloc_tile_pool(name="pss", bufs=1, space="PSUM")
            psp = tc.alloc_tile_pool(name="psp", bufs=1, space="PSUM")

            def conv_group(g):
                """Generator: conv + framing for rows 4g..4g+4; yields per strip."""
                for si, sw in enumerate(STRIPS):
                    s0 = 2048 * si
                    hk = hkp.tile([128, 2112], dt.float32, tag="hk", name="hk")
                    # Hankel: hk[r*32+k, j] = apad[4g+r, s0 + j + k]
                    for r in range(4):
                        srcr = bass.AP(apad, (4 * g + r) * NPAD + s0,
                                       [[1, 32], [1, sw + 32]])
                        nc.sync.dma_start(out=hk[32 * r:32 * r + 32, 0:sw + 32],
                                          in_=srcr)
                    nb4 = (sw + 511) // 512
                    accs = []
                    for b4 in range(nb4):
                        w = min(512, sw - 512 * b4)
                        acc = psp.tile([128, 512], dt.float32, tag=f"acc{b4}",
                                       name="acc")
                        accs.append((acc, w))
                        nc.tensor.matmul(acc[:, 0:w], l1t[:, :],
                                         hk[:, 512 * b4:512 * b4 + w],
                                         start=True, stop=False)
                    for b4 in range(nb4):
                        acc, w = accs[b4]
                        nc.tensor.matmul(acc[:, 0:w], l2t[:, :],
                                         hk[:, 512 * b4 + 32:512 * b4 + 32 + w],
                                         start=False, stop=True)
                    for b4 in range(nb4):
                        acc, w = accs[b4]
                        yb = ybp.tile([128, 512], dt.float32, tag="yb", name="yb")
                        nc.scalar.activation(yb[:, 0:w], acc[:, 0:w], AF.Relu)
                        nblk = w // 128
                        i = 4 * si + b4
                        view = bass.AP(yb.tensor, yb.offset,
                                       [list(yb.ap[0]), [128, nblk], [1, 128]])
                        nc.vector.tensor_reduce(
                            S_all[:, g * 126 + 4 * i: g * 126 + 4 * i + nblk],
                            view, axis=mybir.AxisListType.X, op=OP.add)
                    yield

            def conv_epilogue(g):
                # env[t] = (S[t] + S[t+1]) * (1/256)
                sg = g * 126
                eg = g * T
                nc.vector.tensor_tensor(env_all[:, eg:eg + T],
                                        S_all[:, sg:sg + T],
                                        S_all[:, sg + 1:sg + T + 1], OP.add)
                nc.vector.tensor_scalar(env_all[:, eg:eg + T],
                                        env_all[:, eg:eg + T],
                                        1.0 / 256.0, None, OP.mult)
                # shuffle+replicate env -> E4[u*32+c, (4g+r)*124 + t]
                # via 0/1 selector matmuls (no DMA in the dependency chain)
                shf = pss.tile([128, GF], dt.float32, tag="misc", bufs=2, name="shf")
                for r in range(4):
                    nc.tensor.matmul(shf[:, r * T:(r + 1) * T], selt[r][:, :],
                                     env_all[:, eg:eg + T],
                                     start=True, stop=True)
                nc.scalar.activation(E4[:, g * GF:(g + 1) * GF], shf[:, :],
                                     AF.Copy)

            def an_group(g):
                """AN + bushy currents for group g (columns g*496..)."""
                sl = slice(g * GF, (g + 1) * GF)
                ps_cb = pss.tile([HID, GF], dt.float32, tag="misc", bufs=2, name="ps_cb")
                for ch in range(3):
                    an = anp.tile([128, GF], dt.float32, tag="an", name="an")
                    nc.vector.tensor_scalar(an[:, :], E4[:, sl],
                                            svt[:, ch:ch + 1], AN_THR,
                                            OP.mult, OP.is_gt)
                    nc.tensor.matmul(ps_cb[:, :], wbt[ch][:, :], an[:, :],
                                     start=(ch == 0), stop=(ch == 2))
                nc.scalar.activation(cur_b[:, sl], ps_cb[:, :], AF.Copy)

            cur_b = cp.tile([HID, FREE], dt.float32,
                            padded_shape=[HID, FREE + 32])

            # -------- conv + AN driver (AN one group late to hide deps) --------
            for g in range(NGRP):
                for _ in conv_group(g):
                    pass
                if g >= 1:
                    an_group(g - 1)
                conv_epilogue(g)
            an_group(NGRP - 1)
            psp.release()

            # -------- wavefront LIF: bushy(t), ic(t-1), ac(t-2) --------
            # free layout is b-major: column b*124 + t; a time-slice is
            # a stride-124 AP of 32 columns.
            def tsl(ap2d, t):
                return bass.AP(ap2d.tensor, ap2d.offset + t,
                               [list(ap2d.ap[0]), [T, BLOC]])

            memb = sp.tile([HID, BLOC], dt.float32)
            memic = sp.tile([HID, BLOC], dt.float32)
            nc.vector.memset(memb[:, :], 0.0)
            nc.vector.memset(memic[:, :], 0.0)
            z10 = sp.tile([OUT, BLOC], dt.float32)
            nc.vector.memset(z10[:, :], 0.0)

            spk_b_t = {}
            spk_ic_t = {}
            cur_ic_t = {}
            cur_ac_t = {}

            def ic_mm(t):
                sb = spk_b_t.pop(t)
                pi = pss.tile([HID, BLOC], dt.float32, tag="pp", bufs=2,
                              name="pic")
                nc.tensor.matmul(pi[:, :], wict[:, :], sb[:, :],
                                 start=True, stop=True)
                ci = sp.tile([HID, BLOC], dt.float32, tag="cit", bufs=4,
                             name="cit")
                cur_ic_t[t] = ci
                nc.scalar.activation(ci[:, :], pi[:, :], AF.Copy)

            def ac_mm(t):
                si = spk_ic_t.pop(t)
                pa = pss.tile([OUT, BLOC], dt.float32, tag="pp", bufs=2,
                              name="pac")
                nc.tensor.matmul(pa[:, :], wact[:, :], si[:, :],
                                 start=True, stop=True)
                ca = sp.tile([OUT, BLOC], dt.float32, tag="cat", bufs=4,
                             name="cat")
                cur_ac_t[t] = ca
                nc.scalar.activation(ca[:, :], pa[:, :], AF.Copy)

            # interleave the three chains op-by-op so adjacent DVE ops are
            # from different (independent) chains
            def chain_steps(fns):
                its = [iter(f) for f in fns]
                done = [False] * len(its)
                while not all(done):
                    for k, it in enumerate(its):
                        if not done[k]:
                            try:
                                next(it)
                            except StopIteration:
                                done[k] = True

            def bushy_chain():
                for t in range(T):
                    nc.vector.tensor_scalar(memb[:, :], memb[:, :], BETA,
                                            None, OP.mult)
                    yield
                    nc.vector.tensor_tensor(memb[:, :], memb[:, :],
                                            tsl(cur_b, t), OP.add)
                    yield
                    sb = sp.tile([HID, BLOC], dt.float32, tag="sbt", bufs=4,
                                 name="sbt")
                    spk_b_t[t] = sb
                    nc.vector.tensor_scalar(sb[:, :], memb[:, :], THR,
                                            None, OP.is_gt)
                    ic_mm(t)
                    yield
                    nc.vector.tensor_tensor(memb[:, :], memb[:, :], sb[:, :],
                                            OP.subtract)
                    yield

            def ic_chain():
                yield  # offset by one wavefront step
                for _ in range(4):
                    yield
                for t in range(T):
                    ci = cur_ic_t.pop(t)
                    nc.vector.tensor_scalar(memic[:, :], memic[:, :], BETA,
                                            None, OP.mult)
                    yield
                    nc.vector.tensor_tensor(memic[:, :], memic[:, :],
                                            ci[:, :], OP.add)
                    yield
                    si = sp.tile([HID, BLOC], dt.float32, tag="sit", bufs=4,
                                 name="sit")
                    spk_ic_t[t] = si
                    nc.vector.tensor_scalar(si[:, :], memic[:, :], THR,
                                            None, OP.is_gt)
                    ac_mm(t)
                    yield
                    nc.vector.tensor_tensor(memic[:, :], memic[:, :],
                                            si[:, :], OP.subtract)
                    yield

            def ac_chain():
                for _ in range(8):
                    yield
                for t in range(T):
                    ca = cur_ac_t.pop(t)
                    prev = z10[:, :] if t == 0 else tsl(omem_t, t - 1)
                    nc.vector.tensor_scalar(tsl(omem_t, t), prev, BETA,
                                            None, OP.mult)
                    yield
                    nc.vector.tensor_tensor(tsl(omem_t, t), tsl(omem_t, t),
                                            ca[:, :], OP.add)
                    yield
                    nc.vector.tensor_scalar(tsl(ospk_t, t), tsl(omem_t, t),
                                            THR, None, OP.is_gt)
                    yield
                    nc.vector.tensor_tensor(tsl(omem_t, t), tsl(omem_t, t),
                                            tsl(ospk_t, t), OP.subtract)
                    yield

            chain_steps([bushy_chain(), ic_chain(), ac_chain()])

            nc.sync.dma_start(out=ospk[:, :], in_=ospk_t[:, :])
            nc.sync.dma_start(out=omem[:, :], in_=omem_t[:, :])

            pss.release()
            sp.release()
            anp.release()
            ybp.release()
            hkp.release()

    nc.finalize()
    return nc


def _prep_inputs(audio, gt_kernels, W_bushy, W_ic, W_ac):
    audio = np.ascontiguousarray(audio, dtype=np.float32)
    gt = np.ascontiguousarray(gt_kernels, dtype=np.float32)
    Wb = np.ascontiguousarray(W_bushy, dtype=np.float32)

    l1 = np.zeros((128, 128), np.float32)
    l2 = np.zeros((128, 128), np.float32)
    for r in range(4):
        # lhsT[r*32+k, r*32+c] = gt[c, k]
        l1[r * 32:r * 32 + 32, r * 32:r * 32 + 32] = gt[:, 0:32].T
        l2[r * 32:r * 32 + 32, r * 32:r * 32 + 32] = gt[:, 32:64].T

    wb = np.zeros((3, 128, HID), np.float32)
    sv = np.zeros((128, 3), np.float32)
    for ch in range(3):
        for u in range(4):
            a = ch * 4 + u
            if a >= ANS:
                continue
            # wb[ch, u*32+c, h] = W_bushy[h, c*10 + a]
            wb[ch, u * 32:u * 32 + 32, :] = Wb[:, a::ANS].T
            sv[u * 32:u * 32 + 32, ch] = _SCALES[a]
    selr = np.zeros((4, 128, 128), np.float32)
    for r in range(4):
        for u in range(4):
            for c in range(32):
                selr[r, r * 32 + c, u * 32 + c] = 1.0
    wic = np.ascontiguousarray(W_ic.T, dtype=np.float32)
    wac = np.ascontiguousarray(W_ac.T, dtype=np.float32)

    in_maps = []
    for c in range(NCORES):
        rows = audio[c * BLOC:(c + 1) * BLOC]
        apad = np.zeros((BLOC, NPAD), np.float32)
        apad[:, PAD_L:PAD_L + N] = rows
        in_maps.append({"apad": apad, "l1": l1, "l2": l2, "wb": wb,
                        "wic": wic, "wac": wac, "sv": sv, "selr": selr})
    return in_maps


def kernel(audio, gt_kernels, W_bushy, W_ic, W_ac, _trace=False):
    global _NC_CACHE
    if _NC_CACHE is None:
        _NC_CACHE = _build_nc()
    nc = _NC_CACHE
    in_maps = _prep_inputs(audio, gt_kernels, W_bushy, W_ic, W_ac)
    res = run_bass_kernel_spmd(nc, in_maps, core_ids=list(range(NCORES)),
                               trace=_trace)
    spk = np.empty((B, T, OUT), np.float32)
    mem = np.empty((B, T, OUT), np.float32)
    for c in range(NCORES):
        # [o, b*124+t] -> [b, t, o]
        spk[c * BLOC:(c + 1) * BLOC] = (
            res.results[c]["ospk"].reshape(OUT, BLOC, T).transpose(1, 2, 0))
        mem[c * BLOC:(c + 1) * BLOC] = (
            res.results[c]["omem"].reshape(OUT, BLOC, T).transpose(1, 2, 0))
    kernel._last_results = res
    return spk, mem
